# revision 4
# baseline (speedup 1.0000x reference)
"""BlockSparseLocallyConnected forward on 8 Trainium2 NeuronCores.

Data-parallel over batch: 8 images per core, weights replicated.

out[b, nr, nc] = sum_{dr,dc} xpad[b, 16*nr+dr, 16*nc+dc] * w[(nr,nc), dr*32+dc] + bias

Decomposition: dr = 16*h + u, dc = 16*i + v (h,i in {0,1}; u,v in [0,16)),
nr = 8*g + j (g in [0,4), j in [0,8)).  Patch row = 128*g + 16*(j+h) + u.
With two row-shifted copies of the padded image (shift 0 / 16 rows), SBUF
partition p = 16*j + u holds exactly the rows needed, for both h values.
Columns 16*(nc+i)+v are free-dim strides (overlapping AP reads).

Per (b, g): DVE tensor_mul (bf16) -> product [128, (h,nc,i,v)=2048].
PE matmul with 0/1 selector lhsT L_g[16j+u, 8g+j] reduces u over partitions
and accumulates (g, h) into PSUM [128, (nc,i,v)], 4 batches per PSUM tile
(col-tile offsets 0/32/64/96).  DVE tensor_reduce(axis=XY) folds (i,v),
then bias add.  All layout shuffles/casts are host-side numpy so every DMA
is a contiguous 1:1 copy.
"""

import os
import sys

sys.path.insert(0, "/opt/trn_rl_repo")

import numpy as np
import ml_dtypes

# ---- problem constants (hardcoded; kernel.py must be self-contained) ----
B = 64            # batch
H = W = 512
PH = PW = 8
FULL = 528        # padded H/W
NKH = NKW = 32    # window grid
NCORES = 8
BL = B // NCORES  # batches per core = 8
G = 4             # window-row groups of 8 (nr = 8g + j)
WAVES = BL // 4   # psum waves per core = 2

BF16 = ml_dtypes.bfloat16

_CACHE = {}

TRACE = False          # test.py sets True to get exec_time_ns
LAST_RESULTS = None    # BassKernelResults of last run (for test.py)


def _build_program():
    import concourse.bass as bass
    import concourse.bacc as bacc
    import concourse.tile as tile
    from concourse import mybir

    dt_c = mybir.dt.bfloat16
    f32 = mybir.dt.float32

    # Bacc (not plain Bass): its compile() runs generate_event_semaphores,
    # which splits multi-wait instructions (TRN2 allows 1 wait/instruction).
    nc = bacc.Bacc(
        "TRN2", target_bir_lowering=False, debug=False, num_devices=NCORES
    )
    xs = nc.dram_tensor("xs", [BL, 128, 2, G, FULL], dt_c, kind="ExternalInput")
    wp = nc.dram_tensor("wp", [128, G, 2, 32, 2, 16], dt_c, kind="ExternalInput")
    lm = nc.dram_tensor("lm", [128, G, 32], dt_c, kind="ExternalInput")
    bp = nc.dram_tensor("bp", [128, 32], f32, kind="ExternalInput")
    out_d = nc.dram_tensor("out", [WAVES, 128, 32], f32, kind="ExternalOutput")

    with tile.TileContext(nc) as tc:
        with (
            tc.tile_pool(name="xpool", bufs=BL) as xpool,
            tc.tile_pool(name="cst", bufs=1) as cst,
            tc.tile_pool(name="ppool", bufs=4) as ppool,
            tc.tile_pool(name="psum", bufs=2, space="PSUM") as psum,
            tc.tile_pool(name="opool", bufs=4) as opool,
        ):
            # weights as 4 per-g tiles so the first product only waits on g=0
            w_sb = []
            for g in range(G):
                t = cst.tile([128, 2, 32, 2, 16], dt_c, tag=f"w{g}")
                nc.sync.dma_start(out=t[:], in_=wp[:, g])
                w_sb.append(t)
            l_sb = cst.tile([128, G, 32], dt_c)
            nc.sync.dma_start(out=l_sb[:], in_=lm[:])
            b_sb = cst.tile([128, 32], f32)
            nc.sync.dma_start(out=b_sb[:], in_=bp[:])

            x_sb = []
            for b in range(BL):
                t = xpool.tile([128, 2, G, FULL], dt_c, tag="xb")
                nc.sync.dma_start(out=t[:], in_=xs[b])
                x_sb.append(t)

            psum_t = None
            for b in range(BL):
                wv, c = divmod(b, 4)
                if c == 0:
                    psum_t = psum.tile([128, 32, 2, 16], f32, tag="acc")
                for g in range(G):
                    # overlapping view: [p, (shift, nc, i, v)], col = 16(nc+i)+v
                    base = x_sb[b][:, :, g, :]
                    xview = bass.AP(
                        tensor=base.tensor,
                        offset=base.offset,
                        ap=[
                            list(base.ap[0]),       # partition [step, 128]
                            list(base.ap[1]),       # shift    [2*G*FULL?, 2]
                            [16, 32],               # nc
                            [16, 2],                # i
                            [1, 16],                # v
                        ],
                    )
                    prod = ppool.tile([128, 2, 32, 2, 16], dt_c, tag="prod")
                    nc.vector.tensor_mul(prod[:], xview, w_sb[g][:])
                    for s in range(2):
                        for ch in range(2):
                            nc.tensor.matmul(
                                psum_t[32 * c : 32 * c + 32, 16 * ch : 16 * ch + 16, :, :],
                                l_sb[:, g, :],
                                prod[:, s, 16 * ch : 16 * ch + 16, :, :],
                                start=(g == 0 and s == 0),
                                stop=(g == G - 1 and s == 1),
                                tile_position=(0, 32 * c),
                            )
                if c == 3:
                    tmp = opool.tile([128, 32], f32, tag="tmp")
                    nc.vector.tensor_reduce(
                        tmp[:], psum_t[:],
                        axis=mybir.AxisListType.XY, op=mybir.AluOpType.add,
                    )
                    ow = opool.tile([128, 32], f32, tag="ow")
                    nc.vector.tensor_add(ow[:], tmp[:], b_sb[:])
                    nc.sync.dma_start(out=out_d[wv], in_=ow[:])
    nc.compile()
    return nc


def _prep_inputs(x, weight, bias):
    """Host-side packing: pad, row-shift duplicate, (j,u)-major weight shuffle,
    bf16 cast.  Returns per-core in_maps."""
    x = np.asarray(x, dtype=np.float32)
    weight = np.asarray(weight, dtype=np.float32)
    bias = np.asarray(bias, dtype=np.float32)

    xp = np.zeros((B, FULL, FULL), dtype=np.float32)
    xp[:, PH : PH + H, PW : PW + W] = x[:, 0]
    a = xp[:, 0:512, :].reshape(B, G, 128, FULL)
    bshift = xp[:, 16:528, :].reshape(B, G, 128, FULL)
    # [B, 2, G, 128, FULL] -> [B, 128, 2, G, FULL]
    xs = np.stack([a, bshift], axis=1).transpose(0, 3, 1, 2, 4)
    xs = np.ascontiguousarray(xs).astype(BF16)

    # weight[(8g+j)*32+nc, (16h+u)*32+16i+v] -> wp[16j+u, g, h, nc, i, v]
    wr = weight.reshape(G, 8, 32, 2, 16, 2, 16)          # (g, j, nc, h, u, i, v)
    wp = wr.transpose(1, 4, 0, 3, 2, 5, 6)               # (j, u, g, h, nc, i, v)
    wp = np.ascontiguousarray(wp.reshape(128, G, 2, 32, 2, 16)).astype(BF16)

    # selector matrices: L[16j+u, g, 8g+j] = 1
    lmat = np.zeros((128, G, 32), dtype=np.float32)
    jj = np.arange(8)
    for g in range(G):
        for j in range(8):
            lmat[16 * j : 16 * j + 16, g, 8 * g + j] = 1.0
    lm = lmat.astype(BF16)

    bpk = np.ascontiguousarray(np.tile(bias.reshape(32, 32), (4, 1)))  # [128, 32]

    in_maps = []
    for k in range(NCORES):
        in_maps.append(
            {
                "xs": np.ascontiguousarray(xs[k * BL : (k + 1) * BL]),
                "wp": wp,
                "lm": lm,
                "bp": bpk,
            }
        )
    return in_maps


def kernel(x, weight, bias):
    global LAST_RESULTS
    from concourse.bass_utils import run_bass_kernel_spmd

    if "nc" not in _CACHE:
        _CACHE["nc"] = _build_program()
    nc = _CACHE["nc"]

    in_maps = _prep_inputs(x, weight, bias)
    res = run_bass_kernel_spmd(
        nc, in_maps, core_ids=list(range(NCORES)), trace=TRACE
    )
    LAST_RESULTS = res
    outs = [r["out"].reshape(BL, NKH, NKW) for r in res.results]
    return np.concatenate(outs, axis=0).astype(np.float32)
